# revision 8
# baseline (speedup 1.0000x reference)
import sys

for _p in ("/opt/trn_rl_repo", "/root/.axon_site/_ro/trn_rl_repo"):
    if _p not in sys.path:
        sys.path.insert(0, _p)

import numpy as np

import concourse.bass as bass
import concourse.mybir as mybir
from concourse import masks, tile
from concourse.bass_utils import run_bass_kernel_spmd
from concourse.vector_clock import ScopedClock

F32 = mybir.dt.float32
F32R = mybir.dt.float32r
BF16 = mybir.dt.bfloat16
AF = mybir.ActivationFunctionType
ALU = mybir.AluOpType

B, N, D, H, HD = 4, 2048, 256, 4, 64
NEG_SLOPE = 0.2
P = 128
NI = N // 2
NT = N // P
KT = D // P
JT = NT
NIB = 2
IBLK = NI // NIB
ISUB = IBLK // P
NCORES = 8


def _patch_tile_drain():
    if getattr(tile.TileContext, "_drain_patched", False):
        return

    def _drain_and_barrier(self, tick_clock, wait_clock):
        nc = self.nc
        drain_inst = nc.sync.drain()
        wait_clock.add_sem_waits(
            drain_inst.ins, ScopedClock({None: tick_clock.global_clock})
        )
        si = drain_inst.ins.sync_info
        waits = list(si.on_wait) if (si and si.on_wait) else []
        if len(waits) > 1:
            ups = list(si.on_update) if (si and si.on_update) else []
            drain_inst.ins.sync_info = mybir.SyncInfo(on_wait=waits[:1], on_update=ups)
            for i in range(1, len(waits)):
                extra = nc.sync.drain()
                extra.ins.sync_info = mybir.SyncInfo(
                    on_wait=waits[i : i + 1], on_update=[]
                )
        nc.all_engine_barrier()
        assert self.sems is not None
        popped = nc._tile_sem_poison_stack.pop()
        assert popped is self._sem_poison
        nc.clear_and_free_semaphores(list(self.sems.allocated().values()))
        nc.all_engine_barrier()

    tile.TileContext._drain_and_barrier = _drain_and_barrier
    tile.TileContext._drain_patched = True


def _split_waits(nc, maxw=1):
    n_split = 0
    for f in nc.m.functions:
        for bb in f.blocks:
            insts = list(bb.instructions)
            out = []
            changed = False
            for inst in insts:
                si = inst.sync_info
                waits = list(si.on_wait) if (si and si.on_wait) else []
                if len(waits) > maxw and inst.engine is not None:
                    changed = True
                    extra, keep = waits[:-maxw], waits[-maxw:]
                    for k in range(0, len(extra), maxw):
                        d = mybir.InstDrain(
                            name=f"{inst.name}-wsplit{k}", ins=[], outs=[]
                        )
                        d.engine = inst.engine
                        d.sync_info = mybir.SyncInfo(
                            on_wait=extra[k : k + maxw], on_update=[]
                        )
                        out.append(d)
                        n_split += 1
                    ups = list(si.on_update) if (si and si.on_update) else []
                    inst.sync_info = mybir.SyncInfo(on_wait=keep, on_update=ups)
                out.append(inst)
            if changed:
                bb.instructions = out
    return n_split


def build_nc():
    _patch_tile_drain()
    nc = bass.Bass("TRN2", target_bir_lowering=False, debug=False)

    xb = nc.dram_tensor("xb", [N, D], F32, kind="ExternalInput")
    xi = nc.dram_tensor("xi", [NI, D], F32, kind="ExternalInput")
    wta = nc.dram_tensor("wta", [D, D + 2 * H], F32, kind="ExternalInput")
    adjs = nc.dram_tensor("adjs", [NI, N], F32, kind="ExternalInput")
    outs = nc.dram_tensor("outs", [NI, D], F32, kind="ExternalOutput")

    WC = D + 2 * H
    HP = H * (HD + 1)

    with tile.TileContext(nc) as tc:
        with (
            tc.tile_pool(name="const", bufs=1) as constp,
            tc.tile_pool(name="big", bufs=1) as bigp,
            tc.tile_pool(name="xload", bufs=6) as xloadp,
            tc.tile_pool(name="rows", bufs=1) as rowsp,
            tc.tile_pool(name="adj", bufs=6) as adjp,
            tc.tile_pool(name="adjt", bufs=17) as adjtp,
            tc.tile_pool(name="vwork", bufs=3) as vp,
            tc.tile_pool(name="ptwork", bufs=17) as ptp,
            tc.tile_pool(name="ostage", bufs=4) as ostagep,
            tc.tile_pool(name="small", bufs=8) as smallp,
            tc.tile_pool(name="psmisc", bufs=2, space="PSUM") as psmisc,
            tc.tile_pool(name="psadjt", bufs=2, space="PSUM") as psadjt,
            tc.tile_pool(name="psout", bufs=1, space="PSUM") as psout,
        ):
            ident = constp.tile([P, P], F32, tag="ident")
            masks.make_identity(nc, ident[:])
            ones1 = constp.tile([1, P], BF16, tag="ones1")
            nc.vector.memset(ones1[:], 1.0)

            wta_sb = [constp.tile([P, WC], F32, tag=f"wta{kt}", name=f"wta_sb{kt}") for kt in range(KT)]
            wta_r = [constp.tile([P, WC], F32R, tag=f"wtar{kt}", name=f"wta_r{kt}") for kt in range(KT)]
            for kt in range(KT):
                nc.sync.dma_start(wta_sb[kt][:], wta[kt * P : (kt + 1) * P, :])
                nc.scalar.activation(wta_r[kt][:], wta_sb[kt][:], AF.Copy)

            xt_sb = bigp.tile([P, KT * N], F32R, tag="xt")
            for ntg in range(NT // 4):
                xts = []
                for q in range(4):
                    t = xloadp.tile([P, D], F32, tag="xtile")
                    nt = ntg * 4 + q
                    nc.sync.dma_start(t[:], xb[nt * P : (nt + 1) * P, :])
                    xts.append(t)
                for kt in range(KT):
                    ps = psmisc.tile([P, 512], F32, tag="ps")
                    for q in range(4):
                        nc.tensor.transpose(
                            ps[:, q * P : (q + 1) * P],
                            xts[q][:, kt * P : (kt + 1) * P],
                            ident[:],
                        )
                    nc.scalar.activation(
                        xt_sb[:, kt * N + ntg * 512 : kt * N + (ntg + 1) * 512],
                        ps[:],
                        AF.Copy,
                    )

            xit_sb = bigp.tile([P, KT * NI], F32R, tag="xit")
            for ig in range(NI // P // 4):
                xts = []
                for q in range(4):
                    t = xloadp.tile([P, D], F32, tag="xtile")
                    it = ig * 4 + q
                    nc.sync.dma_start(t[:], xi[it * P : (it + 1) * P, :])
                    xts.append(t)
                for kt in range(KT):
                    ps = psmisc.tile([P, 512], F32, tag="ps")
                    for q in range(4):
                        nc.tensor.transpose(
                            ps[:, q * P : (q + 1) * P],
                            xts[q][:, kt * P : (kt + 1) * P],
                            ident[:],
                        )
                    nc.scalar.activation(
                        xit_sb[:, kt * NI + ig * 512 : kt * NI + (ig + 1) * 512],
                        ps[:],
                        AF.Copy,
                    )

            hplus = bigp.tile([P, NT * HP], BF16, tag="hplus")
            nc.gpsimd.memset(hplus[:], 1.0)
            f1_sb = bigp.tile([P, NT * H], F32, tag="f1")
            f2_sb = bigp.tile([P, NT * H], F32, tag="f2")
            for nt in range(NT):
                psh = psmisc.tile([P, WC], F32, tag="ps")
                for kt in range(KT):
                    nc.tensor.matmul(
                        psh[:],
                        xt_sb[:, kt * N + nt * P : kt * N + (nt + 1) * P],
                        wta_r[kt][:],
                        start=(kt == 0),
                        stop=(kt == KT - 1),
                    )
                for h in range(H):
                    nc.scalar.activation(
                        hplus[:, nt * HP + h * (HD + 1) : nt * HP + h * (HD + 1) + HD],
                        psh[:, h * HD : (h + 1) * HD],
                        AF.Copy,
                    )
                nc.scalar.activation(
                    f1_sb[:, nt * H : (nt + 1) * H],
                    psh[:, D + H : D + 2 * H],
                    AF.Exp,
                )
                nc.scalar.activation(
                    f2_sb[:, nt * H : (nt + 1) * H],
                    psh[:, D + H : D + 2 * H],
                    AF.Exp,
                    scale=NEG_SLOPE,
                )

            er4 = rowsp.tile([H, NI], BF16, tag="er4")
            for c in range(NI // 512):
                pss = psmisc.tile([H, 512], F32, tag="ps")
                for kt in range(KT):
                    nc.tensor.matmul(
                        pss[:],
                        wta_r[kt][:, D : D + H],
                        xit_sb[:, kt * NI + c * 512 : kt * NI + (c + 1) * 512],
                        start=(kt == 0),
                        stop=(kt == KT - 1),
                    )
                nc.scalar.activation(
                    er4[:, c * 512 : (c + 1) * 512], pss[:], AF.Exp, scale=-0.8
                )
            e2rep = bigp.tile([P, H * NI], BF16, tag="e2rep")
            for h in range(H):
                er0 = rowsp.tile([1, NI], BF16, tag=f"er0_{h}")
                nc.sync.dma_start(er0[:], er4[h : h + 1, :])
                for c in range(NI // 512):
                    psb = psmisc.tile([P, 512], F32, tag="ps")
                    nc.tensor.matmul(
                        psb[:], ones1[:], er0[0:1, c * 512 : (c + 1) * 512]
                    )
                    nc.scalar.activation(
                        e2rep[:, h * NI + c * 512 : h * NI + (c + 1) * 512],
                        psb[:],
                        AF.Copy,
                    )

            pe_prev = [None]

            def pe(bi):
                if pe_prev[0] is not None:
                    tile.add_dep_helper(bi.ins, pe_prev[0], reason="pe-order")
                pe_prev[0] = bi.ins
                return bi

            for ib in range(NIB):
                adj_tiles = []
                for isub in range(ISUB):
                    t = adjp.tile([P, N], F32, tag="adjtile", name=f"adj_{ib}_{isub}")
                    r0_ = (ib * ISUB + isub) * P
                    nc.sync.dma_start(t[:], adjs[r0_ : r0_ + P, :])
                    adj_tiles.append(t)
                adjts = []
                for jt in range(JT):
                    pst = psadjt.tile([P, IBLK], F32, tag="pst", name=f"pst_{ib}_{jt}")
                    for isub in range(ISUB):
                        pe(nc.tensor.transpose(
                            pst[:, isub * P : (isub + 1) * P],
                            adj_tiles[isub][:, jt * P : (jt + 1) * P],
                            ident[:],
                        ))
                    adjt = adjtp.tile([P, IBLK], BF16, tag="adjt", name=f"adjt_{ib}_{jt}")
                    nc.scalar.activation(adjt[:], pst[:], AF.Copy)
                    adjts.append(adjt)
                pso = [psout.tile([P, HP], F32, tag=f"pso{h}", name=f"pso_{ib}_{h}") for h in range(H)]
                for h in range(H):
                    pts = []
                    for jt in range(JT):
                        v = vp.tile([P, IBLK], BF16, tag="v")
                        nc.vector.tensor_scalar(
                            v[:],
                            e2rep[:, h * NI + ib * IBLK : h * NI + (ib + 1) * IBLK],
                            f2_sb[:, jt * H + h : jt * H + h + 1],
                            f1_sb[:, jt * H + h : jt * H + h + 1],
                            ALU.mult,
                            ALU.max,
                        )
                        pt = ptp.tile([P, IBLK], BF16, tag="pt", name=f"pt_{ib}_{h}_{jt}")
                        nc.vector.tensor_tensor(pt[:], v[:], adjts[jt][:], ALU.mult)
                        pts.append(pt)
                    for isub in range(ISUB):
                        for jt in range(JT):
                            pe(nc.tensor.matmul(
                                pso[h][:, isub * (HD + 1) : (isub + 1) * (HD + 1)],
                                pts[jt][:, isub * P : (isub + 1) * P],
                                hplus[:, nt_hp(jt, h) : nt_hp(jt, h) + HD + 1],
                                start=(jt == 0),
                                stop=(jt == JT - 1),
                                skip_group_check=True,
                            ))
                for isub in range(ISUB):
                    ost = ostagep.tile([P, D], F32, tag="ost")
                    for h in range(H):
                        rec = smallp.tile([P, 1], F32, tag="rec")
                        base = isub * (HD + 1)
                        nc.vector.reciprocal(
                            rec[:], pso[h][:, base + HD : base + HD + 1]
                        )
                        nc.scalar.activation(
                            ost[:, h * HD : (h + 1) * HD],
                            pso[h][:, base : base + HD],
                            AF.Copy,
                            scale=rec[:],
                        )
                    r0_ = (ib * ISUB + isub) * P
                    nc.sync.dma_start(outs[r0_ : r0_ + P, :], ost[:])

    _split_waits(nc)
    nc.finalize()
    return nc


def nt_hp(jt, h):
    return jt * (H * (HD + 1)) + h * (HD + 1)


_NC_CACHE = None


def _get_nc():
    global _NC_CACHE
    if _NC_CACHE is None:
        _NC_CACHE = build_nc()
    return _NC_CACHE


def kernel(x, adj, W, a_src, a_dst):
    x = np.ascontiguousarray(x, dtype=np.float32)
    adj = np.ascontiguousarray(adj, dtype=np.float32)
    W = np.ascontiguousarray(W, dtype=np.float32)
    a_src = np.ascontiguousarray(a_src, dtype=np.float32)
    a_dst = np.ascontiguousarray(a_dst, dtype=np.float32)

    A_src = np.zeros((D, H), np.float32)
    A_dst = np.zeros((D, H), np.float32)
    for h in range(H):
        A_src[h * HD : (h + 1) * HD, h] = a_src[h]
        A_dst[h * HD : (h + 1) * HD, h] = a_dst[h]
    Wt = W.T.astype(np.float32)
    wta = np.concatenate([Wt, Wt @ A_src, Wt @ A_dst], axis=1)
    wta = np.ascontiguousarray(wta, dtype=np.float32)

    in_maps = []
    for c in range(NCORES):
        b, ihalf = c // 2, c % 2
        ilo = ihalf * NI
        in_maps.append(
            {
                "xb": np.ascontiguousarray(x[b]),
                "xi": np.ascontiguousarray(x[b, ilo : ilo + NI, :]),
                "wta": wta,
                "adjs": np.ascontiguousarray(adj[b, ilo : ilo + NI, :]),
            }
        )

    nc = _get_nc()
    res = run_bass_kernel_spmd(nc, in_maps, list(range(NCORES)))

    out = np.empty((B, N, D), np.float32)
    for c in range(NCORES):
        b, ihalf = c // 2, c % 2
        ilo = ihalf * NI
        out[b, ilo : ilo + NI, :] = res.results[c]["outs"]
    return out


# revision 9
# speedup vs baseline: 1.0817x; 1.0817x over previous
import sys

for _p in ("/opt/trn_rl_repo", "/root/.axon_site/_ro/trn_rl_repo"):
    if _p not in sys.path:
        sys.path.insert(0, _p)

import numpy as np

import concourse.bass as bass
import concourse.mybir as mybir
from concourse import masks, tile
from concourse.bass_utils import run_bass_kernel_spmd
from concourse.vector_clock import ScopedClock

F32 = mybir.dt.float32
F32R = mybir.dt.float32r
BF16 = mybir.dt.bfloat16
AF = mybir.ActivationFunctionType
ALU = mybir.AluOpType

B, N, D, H, HD = 4, 2048, 256, 4, 64
NEG_SLOPE = 0.2
P = 128
NI = N // 2
NT = N // P
KT = D // P
JT = NT
NIB = 2
IBLK = NI // NIB
ISUB = IBLK // P
NCORES = 8


def _patch_tile_drain():
    if getattr(tile.TileContext, "_drain_patched", False):
        return

    def _drain_and_barrier(self, tick_clock, wait_clock):
        nc = self.nc
        drain_inst = nc.sync.drain()
        wait_clock.add_sem_waits(
            drain_inst.ins, ScopedClock({None: tick_clock.global_clock})
        )
        si = drain_inst.ins.sync_info
        waits = list(si.on_wait) if (si and si.on_wait) else []
        if len(waits) > 1:
            ups = list(si.on_update) if (si and si.on_update) else []
            drain_inst.ins.sync_info = mybir.SyncInfo(on_wait=waits[:1], on_update=ups)
            for i in range(1, len(waits)):
                extra = nc.sync.drain()
                extra.ins.sync_info = mybir.SyncInfo(
                    on_wait=waits[i : i + 1], on_update=[]
                )
        nc.all_engine_barrier()
        assert self.sems is not None
        popped = nc._tile_sem_poison_stack.pop()
        assert popped is self._sem_poison
        nc.clear_and_free_semaphores(list(self.sems.allocated().values()))
        nc.all_engine_barrier()

    tile.TileContext._drain_and_barrier = _drain_and_barrier
    tile.TileContext._drain_patched = True


def _split_waits(nc, maxw=1):
    n_split = 0
    for f in nc.m.functions:
        for bb in f.blocks:
            insts = list(bb.instructions)
            out = []
            changed = False
            for inst in insts:
                si = inst.sync_info
                waits = list(si.on_wait) if (si and si.on_wait) else []
                if len(waits) > maxw and inst.engine is not None:
                    changed = True
                    extra, keep = waits[:-maxw], waits[-maxw:]
                    for k in range(0, len(extra), maxw):
                        d = mybir.InstEventSemaphore(
                            name=f"{inst.name}-wsplit{k}", ins=[], outs=[]
                        )
                        d.engine = inst.engine
                        d.sync_info = mybir.SyncInfo(
                            on_wait=extra[k : k + maxw], on_update=[]
                        )
                        out.append(d)
                        n_split += 1
                    ups = list(si.on_update) if (si and si.on_update) else []
                    inst.sync_info = mybir.SyncInfo(on_wait=keep, on_update=ups)
                out.append(inst)
            if changed:
                bb.instructions = out
    return n_split


def build_nc():
    _patch_tile_drain()
    nc = bass.Bass("TRN2", target_bir_lowering=False, debug=False)

    xb = nc.dram_tensor("xb", [N, D], F32, kind="ExternalInput")
    xi = nc.dram_tensor("xi", [NI, D], F32, kind="ExternalInput")
    wta = nc.dram_tensor("wta", [D, D + 2 * H], F32, kind="ExternalInput")
    adjs = nc.dram_tensor("adjs", [NI, N], F32, kind="ExternalInput")
    outs = nc.dram_tensor("outs", [NI, D], F32, kind="ExternalOutput")

    WC = D + 2 * H
    HP = H * (HD + 1)

    with tile.TileContext(nc) as tc:
        with (
            tc.tile_pool(name="const", bufs=1) as constp,
            tc.tile_pool(name="big", bufs=1) as bigp,
            tc.tile_pool(name="xload", bufs=6) as xloadp,
            tc.tile_pool(name="rows", bufs=1) as rowsp,
            tc.tile_pool(name="adj", bufs=6) as adjp,
            tc.tile_pool(name="adjt", bufs=17) as adjtp,
            tc.tile_pool(name="vwork", bufs=3) as vp,
            tc.tile_pool(name="ptwork", bufs=4) as ptp,
            tc.tile_pool(name="ostage", bufs=4) as ostagep,
            tc.tile_pool(name="small", bufs=8) as smallp,
            tc.tile_pool(name="psmisc", bufs=2, space="PSUM") as psmisc,
            tc.tile_pool(name="psadjt", bufs=2, space="PSUM") as psadjt,
            tc.tile_pool(name="psout", bufs=1, space="PSUM") as psout,
        ):
            ident = constp.tile([P, P], F32, tag="ident")
            masks.make_identity(nc, ident[:])
            ones1 = constp.tile([1, P], BF16, tag="ones1")
            nc.vector.memset(ones1[:], 1.0)

            wta_sb = [constp.tile([P, WC], F32, tag=f"wta{kt}", name=f"wta_sb{kt}") for kt in range(KT)]
            wta_r = [constp.tile([P, WC], F32R, tag=f"wtar{kt}", name=f"wta_r{kt}") for kt in range(KT)]
            for kt in range(KT):
                nc.sync.dma_start(wta_sb[kt][:], wta[kt * P : (kt + 1) * P, :])
                nc.scalar.activation(wta_r[kt][:], wta_sb[kt][:], AF.Copy)

            xt_sb = bigp.tile([P, KT * N], F32R, tag="xt")
            for ntg in range(NT // 4):
                xts = []
                for q in range(4):
                    t = xloadp.tile([P, D], F32, tag="xtile")
                    nt = ntg * 4 + q
                    nc.sync.dma_start(t[:], xb[nt * P : (nt + 1) * P, :])
                    xts.append(t)
                for kt in range(KT):
                    ps = psmisc.tile([P, 512], F32, tag="ps")
                    for q in range(4):
                        nc.tensor.transpose(
                            ps[:, q * P : (q + 1) * P],
                            xts[q][:, kt * P : (kt + 1) * P],
                            ident[:],
                        )
                    nc.scalar.activation(
                        xt_sb[:, kt * N + ntg * 512 : kt * N + (ntg + 1) * 512],
                        ps[:],
                        AF.Copy,
                    )

            xit_sb = bigp.tile([P, KT * NI], F32R, tag="xit")
            for ig in range(NI // P // 4):
                xts = []
                for q in range(4):
                    t = xloadp.tile([P, D], F32, tag="xtile")
                    it = ig * 4 + q
                    nc.sync.dma_start(t[:], xi[it * P : (it + 1) * P, :])
                    xts.append(t)
                for kt in range(KT):
                    ps = psmisc.tile([P, 512], F32, tag="ps")
                    for q in range(4):
                        nc.tensor.transpose(
                            ps[:, q * P : (q + 1) * P],
                            xts[q][:, kt * P : (kt + 1) * P],
                            ident[:],
                        )
                    nc.scalar.activation(
                        xit_sb[:, kt * NI + ig * 512 : kt * NI + (ig + 1) * 512],
                        ps[:],
                        AF.Copy,
                    )

            hplus = bigp.tile([P, NT * HP], BF16, tag="hplus")
            nc.gpsimd.memset(hplus[:], 1.0)
            f1_sb = bigp.tile([P, NT * H], F32, tag="f1")
            f2_sb = bigp.tile([P, NT * H], F32, tag="f2")
            for nt in range(NT):
                psh = psmisc.tile([P, WC], F32, tag="ps")
                for kt in range(KT):
                    nc.tensor.matmul(
                        psh[:],
                        xt_sb[:, kt * N + nt * P : kt * N + (nt + 1) * P],
                        wta_r[kt][:],
                        start=(kt == 0),
                        stop=(kt == KT - 1),
                    )
                for h in range(H):
                    nc.scalar.activation(
                        hplus[:, nt * HP + h * (HD + 1) : nt * HP + h * (HD + 1) + HD],
                        psh[:, h * HD : (h + 1) * HD],
                        AF.Copy,
                    )
                nc.scalar.activation(
                    f1_sb[:, nt * H : (nt + 1) * H],
                    psh[:, D + H : D + 2 * H],
                    AF.Exp,
                )
                nc.scalar.activation(
                    f2_sb[:, nt * H : (nt + 1) * H],
                    psh[:, D + H : D + 2 * H],
                    AF.Exp,
                    scale=NEG_SLOPE,
                )

            er4 = rowsp.tile([H, NI], BF16, tag="er4")
            for c in range(NI // 512):
                pss = psmisc.tile([H, 512], F32, tag="ps")
                for kt in range(KT):
                    nc.tensor.matmul(
                        pss[:],
                        wta_r[kt][:, D : D + H],
                        xit_sb[:, kt * NI + c * 512 : kt * NI + (c + 1) * 512],
                        start=(kt == 0),
                        stop=(kt == KT - 1),
                    )
                nc.scalar.activation(
                    er4[:, c * 512 : (c + 1) * 512], pss[:], AF.Exp, scale=-0.8
                )
            e2rep = bigp.tile([P, H * NI], BF16, tag="e2rep")
            for h in range(H):
                er0 = rowsp.tile([1, NI], BF16, tag=f"er0_{h}")
                nc.sync.dma_start(er0[:], er4[h : h + 1, :])
                for c in range(NI // 512):
                    psb = psmisc.tile([P, 512], F32, tag="ps")
                    nc.tensor.matmul(
                        psb[:], ones1[:], er0[0:1, c * 512 : (c + 1) * 512]
                    )
                    nc.scalar.activation(
                        e2rep[:, h * NI + c * 512 : h * NI + (c + 1) * 512],
                        psb[:],
                        AF.Copy,
                    )

            pe_prev = [None]

            def pe(bi):
                if pe_prev[0] is not None:
                    tile.add_dep_helper(bi.ins, pe_prev[0], reason="pe-order")
                pe_prev[0] = bi.ins
                return bi

            for ib in range(NIB):
                adj_tiles = []
                for isub in range(ISUB):
                    t = adjp.tile([P, N], F32, tag="adjtile", name=f"adj_{ib}_{isub}")
                    r0_ = (ib * ISUB + isub) * P
                    nc.sync.dma_start(t[:], adjs[r0_ : r0_ + P, :])
                    adj_tiles.append(t)
                adjts = []
                for jt in range(JT):
                    pst = psadjt.tile([P, IBLK], F32, tag="pst", name=f"pst_{ib}_{jt}")
                    for isub in range(ISUB):
                        pe(nc.tensor.transpose(
                            pst[:, isub * P : (isub + 1) * P],
                            adj_tiles[isub][:, jt * P : (jt + 1) * P],
                            ident[:],
                        ))
                    adjt = adjtp.tile([P, IBLK], BF16, tag="adjt", name=f"adjt_{ib}_{jt}")
                    nc.scalar.activation(adjt[:], pst[:], AF.Copy)
                    adjts.append(adjt)
                psoT = [
                    psout.tile([HD + 1, IBLK], F32, tag=f"pso{h}", name=f"psoT_{ib}_{h}")
                    for h in range(H)
                ]
                for h in range(H):
                    for jt in range(JT):
                        v = vp.tile([P, IBLK], BF16, tag="v")
                        nc.vector.tensor_scalar(
                            v[:],
                            e2rep[:, h * NI + ib * IBLK : h * NI + (ib + 1) * IBLK],
                            f2_sb[:, jt * H + h : jt * H + h + 1],
                            f1_sb[:, jt * H + h : jt * H + h + 1],
                            ALU.mult,
                            ALU.max,
                        )
                        pt = ptp.tile([P, IBLK], BF16, tag="pt", name=f"pt_{ib}_{h}_{jt}")
                        nc.vector.tensor_tensor(pt[:], v[:], adjts[jt][:], ALU.mult)
                        pe(nc.tensor.matmul(
                            psoT[h][:],
                            hplus[:, nt_hp(jt, h) : nt_hp(jt, h) + HD + 1],
                            pt[:],
                            start=(jt == 0),
                            stop=(jt == JT - 1),
                            skip_group_check=True,
                        ))
                ost_tiles = [
                    ostagep.tile([P, D], F32, tag="ost", name=f"ost_{ib}_{q}")
                    for q in range(ISUB)
                ]
                for h in range(H):
                    soT = ostagep.tile(
                        [HD + 1, IBLK], F32, tag="soT", name=f"soT_{ib}_{h}"
                    )
                    nc.scalar.activation(soT[:], psoT[h][:], AF.Copy)
                    for isub in range(ISUB):
                        ps2 = psmisc.tile([P, HD + 1], F32, tag="ps", name=f"ps2_{ib}_{h}_{isub}")
                        pe(nc.tensor.transpose(
                            ps2[:],
                            soT[:, isub * P : (isub + 1) * P],
                            ident[0 : HD + 1, 0 : HD + 1],
                        ))
                        rec = smallp.tile([P, 1], F32, tag="rec")
                        nc.vector.reciprocal(rec[:], ps2[:, HD : HD + 1])
                        nc.scalar.activation(
                            ost_tiles[isub][:, h * HD : (h + 1) * HD],
                            ps2[:, 0:HD],
                            AF.Copy,
                            scale=rec[:],
                        )
                for isub in range(ISUB):
                    r0_ = (ib * ISUB + isub) * P
                    nc.sync.dma_start(outs[r0_ : r0_ + P, :], ost_tiles[isub][:])

    _split_waits(nc)
    nc.finalize()
    return nc


def nt_hp(jt, h):
    return jt * (H * (HD + 1)) + h * (HD + 1)


_NC_CACHE = None


def _get_nc():
    global _NC_CACHE
    if _NC_CACHE is None:
        _NC_CACHE = build_nc()
    return _NC_CACHE


def kernel(x, adj, W, a_src, a_dst):
    x = np.ascontiguousarray(x, dtype=np.float32)
    adj = np.ascontiguousarray(adj, dtype=np.float32)
    W = np.ascontiguousarray(W, dtype=np.float32)
    a_src = np.ascontiguousarray(a_src, dtype=np.float32)
    a_dst = np.ascontiguousarray(a_dst, dtype=np.float32)

    A_src = np.zeros((D, H), np.float32)
    A_dst = np.zeros((D, H), np.float32)
    for h in range(H):
        A_src[h * HD : (h + 1) * HD, h] = a_src[h]
        A_dst[h * HD : (h + 1) * HD, h] = a_dst[h]
    Wt = W.T.astype(np.float32)
    wta = np.concatenate([Wt, Wt @ A_src, Wt @ A_dst], axis=1)
    wta = np.ascontiguousarray(wta, dtype=np.float32)

    in_maps = []
    for c in range(NCORES):
        b, ihalf = c // 2, c % 2
        ilo = ihalf * NI
        in_maps.append(
            {
                "xb": np.ascontiguousarray(x[b]),
                "xi": np.ascontiguousarray(x[b, ilo : ilo + NI, :]),
                "wta": wta,
                "adjs": np.ascontiguousarray(adj[b, ilo : ilo + NI, :]),
            }
        )

    nc = _get_nc()
    res = run_bass_kernel_spmd(nc, in_maps, list(range(NCORES)))

    out = np.empty((B, N, D), np.float32)
    for c in range(NCORES):
        b, ihalf = c // 2, c % 2
        ilo = ihalf * NI
        out[b, ilo : ilo + NI, :] = res.results[c]["outs"]
    return out


# revision 11
# speedup vs baseline: 1.3497x; 1.2478x over previous
import sys

for _p in ("/opt/trn_rl_repo", "/root/.axon_site/_ro/trn_rl_repo"):
    if _p not in sys.path:
        sys.path.insert(0, _p)

import ml_dtypes
import numpy as np

import concourse.bass as bass
import concourse.mybir as mybir
from concourse import masks, tile
from concourse.bass_utils import run_bass_kernel_spmd
from concourse.vector_clock import ScopedClock

F32 = mybir.dt.float32
F32R = mybir.dt.float32r
BF16 = mybir.dt.bfloat16
AF = mybir.ActivationFunctionType
ALU = mybir.AluOpType

B, N, D, H, HD = 4, 2048, 256, 4, 64
NEG_SLOPE = 0.2
P = 128
NI = N // 2
NT = N // P
KT = D // P
JT = NT
NIB = 2
IBLK = NI // NIB
ISUB = IBLK // P
NCORES = 8


def _patch_tile_drain():
    if getattr(tile.TileContext, "_drain_patched", False):
        return

    def _drain_and_barrier(self, tick_clock, wait_clock):
        nc = self.nc
        drain_inst = nc.sync.drain()
        wait_clock.add_sem_waits(
            drain_inst.ins, ScopedClock({None: tick_clock.global_clock})
        )
        si = drain_inst.ins.sync_info
        waits = list(si.on_wait) if (si and si.on_wait) else []
        if len(waits) > 1:
            ups = list(si.on_update) if (si and si.on_update) else []
            drain_inst.ins.sync_info = mybir.SyncInfo(on_wait=waits[:1], on_update=ups)
            for i in range(1, len(waits)):
                extra = nc.sync.drain()
                extra.ins.sync_info = mybir.SyncInfo(
                    on_wait=waits[i : i + 1], on_update=[]
                )
        nc.all_engine_barrier()
        assert self.sems is not None
        popped = nc._tile_sem_poison_stack.pop()
        assert popped is self._sem_poison
        nc.clear_and_free_semaphores(list(self.sems.allocated().values()))
        nc.all_engine_barrier()

    tile.TileContext._drain_and_barrier = _drain_and_barrier
    tile.TileContext._drain_patched = True


def _split_waits(nc, maxw=1):
    n_split = 0
    for f in nc.m.functions:
        for bb in f.blocks:
            insts = list(bb.instructions)
            out = []
            changed = False
            for inst in insts:
                si = inst.sync_info
                waits = list(si.on_wait) if (si and si.on_wait) else []
                if len(waits) > maxw and inst.engine is not None:
                    changed = True
                    extra, keep = waits[:-maxw], waits[-maxw:]
                    for k in range(0, len(extra), maxw):
                        d = mybir.InstEventSemaphore(
                            name=f"{inst.name}-wsplit{k}", ins=[], outs=[]
                        )
                        d.engine = inst.engine
                        d.sync_info = mybir.SyncInfo(
                            on_wait=extra[k : k + maxw], on_update=[]
                        )
                        out.append(d)
                        n_split += 1
                    ups = list(si.on_update) if (si and si.on_update) else []
                    inst.sync_info = mybir.SyncInfo(on_wait=keep, on_update=ups)
                out.append(inst)
            if changed:
                bb.instructions = out
    return n_split


def build_nc():
    _patch_tile_drain()
    nc = bass.Bass("TRN2", target_bir_lowering=False, debug=False)

    xb = nc.dram_tensor("xb", [N, D], F32, kind="ExternalInput")
    xi = nc.dram_tensor("xi", [NI, D], F32, kind="ExternalInput")
    wta = nc.dram_tensor("wta", [D, D + 2 * H], F32, kind="ExternalInput")
    adjb = nc.dram_tensor("adjb", [NI, N], BF16, kind="ExternalInput")
    outs = nc.dram_tensor("outs", [NI, D], F32, kind="ExternalOutput")

    WC = D + 2 * H
    HP = H * (HD + 1)

    with tile.TileContext(nc) as tc:
        with (
            tc.tile_pool(name="const", bufs=1) as constp,
            tc.tile_pool(name="big", bufs=1) as bigp,
            tc.tile_pool(name="xload", bufs=6) as xloadp,
            tc.tile_pool(name="rows", bufs=1) as rowsp,
            tc.tile_pool(name="adjt", bufs=17) as adjtp,
            tc.tile_pool(name="vwork", bufs=3) as vp,
            tc.tile_pool(name="ptwork", bufs=4) as ptp,
            tc.tile_pool(name="ostage", bufs=4) as ostagep,
            tc.tile_pool(name="small", bufs=8) as smallp,
            tc.tile_pool(name="psmisc", bufs=4, space="PSUM") as psmisc,
            tc.tile_pool(name="psout", bufs=1, space="PSUM") as psout,
        ):
            ident = constp.tile([P, P], F32, tag="ident")
            masks.make_identity(nc, ident[:])
            ones1 = constp.tile([1, P], BF16, tag="ones1")
            nc.vector.memset(ones1[:], 1.0)

            wta_sb = [constp.tile([P, WC], F32, tag=f"wta{kt}", name=f"wta_sb{kt}") for kt in range(KT)]
            wta_r = [constp.tile([P, WC], F32R, tag=f"wtar{kt}", name=f"wta_r{kt}") for kt in range(KT)]
            for kt in range(KT):
                nc.sync.dma_start(wta_sb[kt][:], wta[kt * P : (kt + 1) * P, :])
                nc.scalar.activation(wta_r[kt][:], wta_sb[kt][:], AF.Copy)

            xt_sb = bigp.tile([P, KT * N], F32R, tag="xt")
            for ntg in range(NT // 4):
                xts = []
                for q in range(4):
                    t = xloadp.tile([P, D], F32, tag="xtile")
                    nt = ntg * 4 + q
                    nc.sync.dma_start(t[:], xb[nt * P : (nt + 1) * P, :])
                    xts.append(t)
                for kt in range(KT):
                    ps = psmisc.tile([P, 512], F32, tag="ps")
                    for q in range(4):
                        nc.tensor.transpose(
                            ps[:, q * P : (q + 1) * P],
                            xts[q][:, kt * P : (kt + 1) * P],
                            ident[:],
                        )
                    nc.scalar.activation(
                        xt_sb[:, kt * N + ntg * 512 : kt * N + (ntg + 1) * 512],
                        ps[:],
                        AF.Copy,
                    )

            xit_sb = bigp.tile([P, KT * NI], F32R, tag="xit")
            for ig in range(NI // P // 4):
                xts = []
                for q in range(4):
                    t = xloadp.tile([P, D], F32, tag="xtile")
                    it = ig * 4 + q
                    nc.sync.dma_start(t[:], xi[it * P : (it + 1) * P, :])
                    xts.append(t)
                for kt in range(KT):
                    ps = psmisc.tile([P, 512], F32, tag="ps")
                    for q in range(4):
                        nc.tensor.transpose(
                            ps[:, q * P : (q + 1) * P],
                            xts[q][:, kt * P : (kt + 1) * P],
                            ident[:],
                        )
                    nc.scalar.activation(
                        xit_sb[:, kt * NI + ig * 512 : kt * NI + (ig + 1) * 512],
                        ps[:],
                        AF.Copy,
                    )

            hplus = bigp.tile([P, NT * HP], BF16, tag="hplus")
            nc.gpsimd.memset(hplus[:], 1.0)
            f1_sb = bigp.tile([P, NT * H], F32, tag="f1")
            f2_sb = bigp.tile([P, NT * H], F32, tag="f2")
            for nt in range(NT):
                psh = psmisc.tile([P, WC], F32, tag="ps")
                for kt in range(KT):
                    nc.tensor.matmul(
                        psh[:],
                        xt_sb[:, kt * N + nt * P : kt * N + (nt + 1) * P],
                        wta_r[kt][:],
                        start=(kt == 0),
                        stop=(kt == KT - 1),
                    )
                for h in range(H):
                    nc.scalar.activation(
                        hplus[:, nt * HP + h * (HD + 1) : nt * HP + h * (HD + 1) + HD],
                        psh[:, h * HD : (h + 1) * HD],
                        AF.Copy,
                    )
                nc.scalar.activation(
                    f1_sb[:, nt * H : (nt + 1) * H],
                    psh[:, D + H : D + 2 * H],
                    AF.Exp,
                )
                nc.scalar.activation(
                    f2_sb[:, nt * H : (nt + 1) * H],
                    psh[:, D + H : D + 2 * H],
                    AF.Exp,
                    scale=NEG_SLOPE,
                )

            er4 = rowsp.tile([H, NI], BF16, tag="er4")
            for c in range(NI // 512):
                pss = psmisc.tile([H, 512], F32, tag="ps")
                for kt in range(KT):
                    nc.tensor.matmul(
                        pss[:],
                        wta_r[kt][:, D : D + H],
                        xit_sb[:, kt * NI + c * 512 : kt * NI + (c + 1) * 512],
                        start=(kt == 0),
                        stop=(kt == KT - 1),
                    )
                nc.scalar.activation(
                    er4[:, c * 512 : (c + 1) * 512], pss[:], AF.Exp, scale=-0.8
                )
            e2rep = bigp.tile([P, H * NI], BF16, tag="e2rep")
            for h in range(H):
                er0 = rowsp.tile([1, NI], BF16, tag=f"er0_{h}")
                nc.sync.dma_start(er0[:], er4[h : h + 1, :])
                for c in range(NI // 512):
                    psb = psmisc.tile([P, 512], F32, tag="ps")
                    nc.tensor.matmul(
                        psb[:], ones1[:], er0[0:1, c * 512 : (c + 1) * 512]
                    )
                    nc.scalar.activation(
                        e2rep[:, h * NI + c * 512 : h * NI + (c + 1) * 512],
                        psb[:],
                        AF.Copy,
                    )

            pe_prev = [None]

            def pe(bi):
                if pe_prev[0] is not None:
                    tile.add_dep_helper(bi.ins, pe_prev[0], reason="pe-order")
                pe_prev[0] = bi.ins
                return bi

            for ib in range(NIB):
                adjts = []
                for jt in range(JT):
                    adjt = adjtp.tile([P, IBLK], BF16, tag="adjt", name=f"adjt_{ib}_{jt}")
                    nc.sync.dma_start_transpose(
                        adjt[:],
                        adjb[ib * IBLK : (ib + 1) * IBLK, jt * P : (jt + 1) * P],
                    )
                    adjts.append(adjt)
                psoT = [
                    psout.tile([HD + 1, IBLK], F32, tag=f"pso{h}", name=f"psoT_{ib}_{h}")
                    for h in range(H)
                ]
                for h in range(H):
                    for jt in range(JT):
                        v = vp.tile([P, IBLK], BF16, tag="v")
                        nc.vector.tensor_scalar(
                            v[:],
                            e2rep[:, h * NI + ib * IBLK : h * NI + (ib + 1) * IBLK],
                            f2_sb[:, jt * H + h : jt * H + h + 1],
                            f1_sb[:, jt * H + h : jt * H + h + 1],
                            ALU.mult,
                            ALU.max,
                        )
                        pt = ptp.tile([P, IBLK], BF16, tag="pt", name=f"pt_{ib}_{h}_{jt}")
                        nc.vector.tensor_tensor(pt[:], v[:], adjts[jt][:], ALU.mult)
                        pe(nc.tensor.matmul(
                            psoT[h][:],
                            hplus[:, nt_hp(jt, h) : nt_hp(jt, h) + HD + 1],
                            pt[:],
                            start=(jt == 0),
                            stop=(jt == JT - 1),
                            skip_group_check=True,
                        ))
                ost_tiles = [
                    ostagep.tile([P, D], F32, tag="ost", name=f"ost_{ib}_{q}")
                    for q in range(ISUB)
                ]
                for h in range(H):
                    soT = ostagep.tile(
                        [HD + 1, IBLK], F32, tag="soT", name=f"soT_{ib}_{h}"
                    )
                    nc.scalar.activation(soT[:], psoT[h][:], AF.Copy)
                    for isub in range(ISUB):
                        ps2 = psmisc.tile([P, HD + 1], F32, tag="ps", name=f"ps2_{ib}_{h}_{isub}")
                        pe(nc.tensor.transpose(
                            ps2[:],
                            soT[:, isub * P : (isub + 1) * P],
                            ident[0 : HD + 1, 0 : HD + 1],
                        ))
                        rec = smallp.tile([P, 1], F32, tag="rec")
                        nc.vector.reciprocal(rec[:], ps2[:, HD : HD + 1])
                        nc.scalar.activation(
                            ost_tiles[isub][:, h * HD : (h + 1) * HD],
                            ps2[:, 0:HD],
                            AF.Copy,
                            scale=rec[:],
                        )
                for isub in range(ISUB):
                    r0_ = (ib * ISUB + isub) * P
                    nc.sync.dma_start(outs[r0_ : r0_ + P, :], ost_tiles[isub][:])

    _split_waits(nc)
    nc.finalize()
    return nc


def nt_hp(jt, h):
    return jt * (H * (HD + 1)) + h * (HD + 1)


_NC_CACHE = None


def _get_nc():
    global _NC_CACHE
    if _NC_CACHE is None:
        _NC_CACHE = build_nc()
    return _NC_CACHE


def kernel(x, adj, W, a_src, a_dst):
    x = np.ascontiguousarray(x, dtype=np.float32)
    adj = np.ascontiguousarray(adj, dtype=np.float32)
    W = np.ascontiguousarray(W, dtype=np.float32)
    a_src = np.ascontiguousarray(a_src, dtype=np.float32)
    a_dst = np.ascontiguousarray(a_dst, dtype=np.float32)

    A_src = np.zeros((D, H), np.float32)
    A_dst = np.zeros((D, H), np.float32)
    for h in range(H):
        A_src[h * HD : (h + 1) * HD, h] = a_src[h]
        A_dst[h * HD : (h + 1) * HD, h] = a_dst[h]
    Wt = W.T.astype(np.float32)
    wta = np.concatenate([Wt, Wt @ A_src, Wt @ A_dst], axis=1)
    wta = np.ascontiguousarray(wta, dtype=np.float32)

    in_maps = []
    for c in range(NCORES):
        b, ihalf = c // 2, c % 2
        ilo = ihalf * NI
        in_maps.append(
            {
                "xb": np.ascontiguousarray(x[b]),
                "xi": np.ascontiguousarray(x[b, ilo : ilo + NI, :]),
                "wta": wta,
                "adjb": np.ascontiguousarray(
                    adj[b, ilo : ilo + NI, :].astype(ml_dtypes.bfloat16)
                ),
            }
        )

    nc = _get_nc()
    res = run_bass_kernel_spmd(nc, in_maps, list(range(NCORES)))

    out = np.empty((B, N, D), np.float32)
    for c in range(NCORES):
        b, ihalf = c // 2, c % 2
        ilo = ihalf * NI
        out[b, ilo : ilo + NI, :] = res.results[c]["outs"]
    return out
